# revision 28
# baseline (speedup 1.0000x reference)
"""Trainium2 Bass kernel for nn_CapsuleLayer (4x 3D capsule convs + dynamic routing).

Self-contained: kernel(**inputs) takes the FULL unsharded inputs and returns the
FULL output. Internally shards the 48^3 volume across 8 NeuronCores along H
(6 output rows each, 1-row halo), runs one SPMD Bass program, and gathers.

Math (exploits the reference's routing list-collapse):
  u_hat_i = relu(conv3d(u[:,i], W[i]) + bias_i)          (4 capsule convs)
  S = sum_i u_hat_i;  per (t, pos): St, SSt, s3, s0, q0  (z1-reductions)
  b1 = f(SSt/64)*St/8*s3 ; c2 = softmax_t(b1)
  b2 = b1 + f(c2^2 q0)*c2*s0^2 ; c3 = softmax_t(b2)
  out[t,z] = f(c3^2 q0)*c3 * u_hat_0[t,z],  f(s) = (s/(1+s))/sqrt(s+1e-9)

Conv layout: per output h-row, the (w,d) plane is zero-padded to 50x50 and
flattened (f = w*50 + d) so all 27 taps are free-axis offsets. Contraction is
stacked to K=128 over 8 (kh,kw) tap-pairs x z0=16 (the 9th pair runs as 3
row-tiled K=16 matmuls), with kd as a free-axis shift. All matmuls fp32
(the routing softmax is chaotic: logits span +-100, so bf16/tf32 convs are
numerically dead).
"""

import os
import numpy as np

import concourse.bass as bass
import concourse.tile as tile
from concourse import bacc
from concourse import mybir
from concourse.bass_utils import run_bass_kernel_spmd

F32 = mybir.dt.float32
AF = mybir.ActivationFunctionType
ALU = mybir.AluOpType

T0, Z0, T1, Z1 = 4, 16, 8, 16
H = W_ = D = 48
NCORES, ROWS = 8, 6          # 8 cores x 6 output h-rows
PW = 50                      # padded w/d extent
PF = 2504                    # padded flattened plane row (50*50 -> 2504)
BW = 2404                    # stacked block free width
ROWF = 2400                  # per-row output f extent (48 w-rows x 50)
TILES = [(0, 500), (500, 500), (1000, 500), (1500, 500), (2000, 400)]
CHUNK, NCHUNK = 120, 20      # transpose chunking of ROWF
PAIRS = [(a, b) for a in range(3) for b in range(3)][:8]   # leftover: (2,2)
# bisect knobs (debug only)
_ROWS_RUN = int(os.environ.get("K_ROWS", ROWS))
_CAPS_RUN = int(os.environ.get("K_CAPS", T0))
_PHASE = int(os.environ.get("K_PHASE", 9))   # 1=conv,2=+reduce,3=+chain,9=all


def _build_program():
    nc = bacc.Bacc("TRN2", target_bir_lowering=False, debug=False)

    u_slab = nc.dram_tensor("u_slab", [T0, Z0, ROWS + 2, PF], F32,
                            kind="ExternalInput").ap()
    wmain = nc.dram_tensor("wmain", [T0, 3, 128, 128], F32,
                           kind="ExternalInput").ap()
    wlft = nc.dram_tensor("wlft", [T0, 48, 128], F32,
                          kind="ExternalInput").ap()
    biasT = nc.dram_tensor("biasT", [128, T0], F32, kind="ExternalInput").ap()
    iden = nc.dram_tensor("iden", [128, 128], F32, kind="ExternalInput").ap()
    out = nc.dram_tensor("out", [128, ROWS, W_, D], F32,
                         kind="ExternalOutput").ap()
    # windowed DRAM staging: u3[cap, z, r, b, f] = u_slab[cap, z, r, b*50+f]
    u3 = nc.dram_tensor("u3", [T0, Z0, ROWS + 2, 3, BW], F32,
                        kind="Internal").ap()

    from contextlib import ExitStack
    with tile.TileContext(nc) as tc, ExitStack() as es:
        const = es.enter_context(tc.tile_pool(name="const", bufs=1))
        stkp = es.enter_context(tc.tile_pool(name="stkp", bufs=3))
        rowp = es.enter_context(tc.tile_pool(name="rowp", bufs=2))
        scrp = es.enter_context(tc.tile_pool(name="scrp", bufs=2))
        chp = es.enter_context(tc.tile_pool(name="chp", bufs=2))
        psc = es.enter_context(tc.tile_pool(name="psc", bufs=2, space="PSUM"))
        pst = es.enter_context(tc.tile_pool(name="pst", bufs=2, space="PSUM"))
        psg = es.enter_context(tc.tile_pool(name="psg", bufs=1, space="PSUM"))
        # dedicated bank for "touch" matmuls that pre-absorb DMA semaphores
        # onto the PE vector clock (the LW+MM pair encodes very few sync
        # commands, so real matmuls must arrive with their waits pre-observed)
        pstc = es.enter_context(tc.tile_pool(name="pstc", bufs=1, space="PSUM"))

        # --- constants ---
        wm = const.tile([128, T0 * 3, 128], F32)          # main lhsT per (cap,c3)
        for cap in range(T0):
            for c3 in range(3):
                nc.sync.dma_start(wm[:, cap * 3 + c3, :], wmain[cap, c3])
        wl = const.tile([48, T0, 128], F32)               # leftover lhsT (c3,z0)
        for cap in range(T0):
            nc.sync.dma_start(wl[:, cap, :], wlft[cap])
        bias = const.tile([128, T0], F32)
        nc.sync.dma_start(bias[:], biasT[:])
        idn = const.tile([128, 128], F32)
        nc.sync.dma_start(idn[:], iden[:])
        epsT = const.tile([128, 1], F32)
        nc.any.memset(epsT[:], 1e-9)

        # DRAM->DRAM window expansion (3-dim APs, one DMA per (cap, row))
        for cap in range(T0):
            for r in range(ROWS + 2):
                s = bass.AP(u_slab.tensor,
                            u_slab.offset + (cap * Z0 * 8 + r) * PF,
                            [[8 * PF, Z0], [PW, 3], [1, BW]])
                nc.sync.dma_start(u3[cap, :, r], s)

        ptc = pstc.tile([1, 16], F32)

        def touch(rhs_ap, lhsT_ap=None):
            nc.tensor.matmul(ptc[0:1, 0:2],
                             lhsT_ap if lhsT_ap is not None else idn[:, 0:1],
                             rhs_ap, start=True, stop=True,
                             skip_group_check=True)

        touch(wm[:, 0, 0:2], wm[:, 0, 0:1])
        touch(idn[:, 0:2], idn[:, 0:1])

        for h in range(_ROWS_RUN):
            U0 = rowp.tile([128, ROWF], F32, tag="U0")
            S = rowp.tile([128, ROWF], F32, tag="S")

            for cap in range(_CAPS_RUN):
                stk = stkp.tile([128, BW], F32, tag="stk")
                # pairs p=3a+b are contiguous (r*3+b = h*3+p) in u3
                s = bass.AP(u3.tensor,
                            u3.offset + (cap * Z0 * (ROWS + 2) * 3 + h * 3) * BW,
                            [[BW, 8], [(ROWS + 2) * 3 * BW, Z0], [1, BW]])
                nc.sync.dma_start(stk[:, :], s)
                lft = stkp.tile([48, BW], F32, tag="lft")
                for c3 in range(3):
                    nc.sync.dma_start(lft[Z0 * c3:Z0 * (c3 + 1), 0:BW - 2],
                                      u3[cap, :, h + 2, 2, c3:c3 + BW - 2])
                touch(stk[:, 0:2])
                touch(lft[:, 0:2], wl[:, cap, 0:1])

                uh = U0 if cap == 0 else scrp.tile([128, ROWF], F32, tag="uh")
                for off, wd in TILES:
                    ps = psc.tile([128, 512], F32, tag="conv")
                    for c3 in range(3):
                        nc.tensor.matmul(ps[:, :wd], wm[:, cap * 3 + c3, :],
                                         stk[:, off + c3:off + c3 + wd],
                                         start=(c3 == 0), stop=False)
                    nc.tensor.matmul(ps[:, :wd], wl[:, cap, :],
                                     lft[:, off:off + wd],
                                     start=False, stop=True)
                    # bias + relu (PSUM -> SBUF)
                    nc.scalar.activation(uh[:, off:off + wd], ps[:, :wd],
                                         AF.Relu, bias=bias[:, cap:cap + 1])

                    if _PHASE < 2 or cap == 0:
                        continue
                    base = U0 if cap == 1 else S
                    nc.vector.tensor_add(S[:, off:off + wd],
                                         base[:, off:off + wd],
                                         uh[:, off:off + wd])
                if cap == 3:
                    uh3 = uh

            if _PHASE < 3:
                nc.sync.dma_start(
                    out[:, h],
                    U0[:].rearrange("p (w d) -> p w d", d=PW)[:, :, 0:D])
                continue
            # --- transpose S/U0/uh3 to spatial-major, reduce z1 on DVE ---
            Rs0 = chp.tile([CHUNK, NCHUNK, 8], F32, tag="Rs0")
            Rq0 = chp.tile([CHUNK, NCHUNK, 8], F32, tag="Rq0")
            Rst = chp.tile([CHUNK, NCHUNK, 8], F32, tag="Rst")
            Rss = chp.tile([CHUNK, NCHUNK, 8], F32, tag="Rss")
            Rs3 = chp.tile([CHUNK, NCHUNK, 8], F32, tag="Rs3")
            for g in range(NCHUNK // 2):          # 2 chunks per group
                tp = pst.tile([CHUNK, 2, 512], F32, tag="tp")
                for j in range(2):
                    c = 2 * g + j
                    sl = slice(CHUNK * c, CHUNK * (c + 1))
                    nc.tensor.transpose(tp[:, j, 0:128], S[:, sl], idn)
                    nc.tensor.transpose(tp[:, j, 128:256], U0[:, sl], idn)
                    nc.tensor.transpose(tp[:, j, 256:384], uh3[:, sl], idn)
                gsl = slice(2 * g, 2 * g + 2)
                qsc = chp.tile([CHUNK, 2, 128], F32, tag="qsc")
                for (R, base) in ((Rst, 0), (Rs0, 128), (Rs3, 256)):
                    nc.vector.tensor_reduce(
                        R[:, gsl, :],
                        tp[:, :, base:base + 128].rearrange(
                            "p g (t z) -> p g t z", z=Z1),
                        mybir.AxisListType.X, ALU.add)
                for (R, base) in ((Rss, 0), (Rq0, 128)):
                    nc.scalar.activation(qsc[:], tp[:, :, base:base + 128],
                                         AF.Square)
                    nc.vector.tensor_reduce(
                        R[:, gsl, :],
                        qsc[:].rearrange("p g (t z) -> p g t z", z=Z1),
                        mybir.AxisListType.X, ALU.add)

            s0v = Rs0[:]
            q0v = Rq0[:]
            s3v = Rs3[:]

            sh8 = [CHUNK, NCHUNK, 8]
            sh1 = [CHUNK, NCHUNK, 1]

            def t8(tag):
                return chp.tile(sh8, F32, tag=tag, name=tag)

            def t1(tag):
                return chp.tile(sh1, F32, tag=tag, name=tag)

            def fchain(sqv, tagp):
                """returns tile containing f(sqv) = sqv/((1+sqv)*sqrt(sqv+1e-9))"""
                r = t8(tagp + "r")
                nc.scalar.activation(r[:], sqv, AF.Sqrt, bias=epsT[0:CHUNK, :])
                d = t8(tagp + "d")
                nc.vector.tensor_scalar_add(d[:], sqv, 1.0)
                nc.vector.tensor_mul(d[:], d[:], r[:])
                rc = t8(tagp + "rc")
                nc.vector.reciprocal(rc[:], d[:])
                f = t8(tagp + "f")
                nc.vector.tensor_mul(f[:], sqv, rc[:])
                return f

            def softmax(b, tagp):
                m = t1(tagp + "m")
                nc.vector.tensor_reduce(m[:], b[:], mybir.AxisListType.X, ALU.max)
                e = t8(tagp + "e")
                nc.vector.tensor_sub(e[:], b[:], m[:].broadcast_to(sh8))
                nc.scalar.activation(e[:], e[:], AF.Exp)
                dn = t1(tagp + "dn")
                nc.vector.tensor_reduce(dn[:], e[:], mybir.AxisListType.X, ALU.add)
                rd = t1(tagp + "rd")
                nc.vector.reciprocal(rd[:], dn[:])
                c = t8(tagp + "c")
                nc.vector.tensor_mul(c[:], e[:], rd[:].broadcast_to(sh8))
                return c

            sq1 = t8("sq1")
            nc.vector.tensor_scalar_mul(sq1[:], Rss[:], 1.0 / 64.0)
            f1 = fchain(sq1[:], "f1")
            b1 = t8("b1")
            nc.vector.scalar_tensor_tensor(b1[:], f1[:], 0.125, Rst[:],
                                           ALU.mult, ALU.mult)
            nc.vector.tensor_mul(b1[:], b1[:], s3v)
            c2 = softmax(b1, "s1")
            sq2 = t8("sq2")
            nc.vector.tensor_mul(sq2[:], c2[:], c2[:])
            nc.vector.tensor_mul(sq2[:], sq2[:], q0v)
            f2 = fchain(sq2[:], "f2")
            nc.vector.tensor_mul(f2[:], f2[:], c2[:])
            nc.vector.tensor_mul(f2[:], f2[:], s0v)
            nc.vector.tensor_mul(f2[:], f2[:], s0v)
            b2 = t8("b2")
            nc.vector.tensor_add(b2[:], b1[:], f2[:])
            c3 = softmax(b2, "s2")
            sq3 = t8("sq3")
            nc.vector.tensor_mul(sq3[:], c3[:], c3[:])
            nc.vector.tensor_mul(sq3[:], sq3[:], q0v)
            g = fchain(sq3[:], "f3")
            nc.vector.tensor_mul(g[:], g[:], c3[:])

            # replicate g over z1: [120, 20, 8] -> [120, 20, 8, 16]
            grep = chp.tile([CHUNK, NCHUNK, 128], F32, tag="grep")
            nc.vector.tensor_copy(
                grep[:].rearrange("p n (t z) -> p n t z", z=Z1),
                g[:].unsqueeze(3).broadcast_to([CHUNK, NCHUNK, 8, Z1]))

            # transpose back per chunk and scale u_hat_0
            stage = rowp.tile([128, ROWF], F32, tag="stage")
            for c in range(NCHUNK):
                gb = psg.tile([128, CHUNK], F32, tag="gb")
                nc.tensor.transpose(gb[:], grep[:, c, :], idn[0:CHUNK, 0:CHUNK])
                nc.vector.tensor_mul(stage[:, CHUNK * c:CHUNK * (c + 1)],
                                     gb[:], U0[:, CHUNK * c:CHUNK * (c + 1)])

            nc.sync.dma_start(
                out[:, h],
                stage[:].rearrange("p (w d) -> p w d", d=PW)[:, :, 0:D])

    nc.compile()
    return nc


def _host_prep(u, W, bias):
    """Returns (per-core in_maps list, shared tensors dict)."""
    u = np.ascontiguousarray(u.astype(np.float32))
    W = np.ascontiguousarray(W.astype(np.float32))
    bias = np.ascontiguousarray(bias.astype(np.float32))

    u_pad = np.zeros((T0, Z0, H + 2, PW, PW), np.float32)
    u_pad[:, :, 1:-1, 1:-1, 1:-1] = u[0]
    u_pad = u_pad.reshape(T0, Z0, H + 2, PW * PW)
    u_padf = np.zeros((T0, Z0, H + 2, PF), np.float32)
    u_padf[:, :, :, :PW * PW] = u_pad

    wmain = np.zeros((T0, 3, 128, 128), np.float32)
    for p, (a, b) in enumerate(PAIRS):
        # lhsT[(16p+z0), co] = W[cap, co, z0, a, b, c3]
        wmain[:, :, Z0 * p:Z0 * (p + 1), :] = W[:, :, :, a, b, :].transpose(
            0, 3, 2, 1)
    # leftover lhsT [(c3, z0), co], c3-major to match the pre-shifted data
    wlft = W[:, :, :, 2, 2, :].transpose(0, 3, 2, 1).reshape(T0, 48, 128).copy()

    biasT = bias.T.copy()                     # [128, T0]
    iden = np.eye(128, dtype=np.float32)

    shared = {"wmain": wmain, "wlft": wlft,
              "biasT": biasT, "iden": iden}
    in_maps = []
    for k in range(NCORES):
        m = dict(shared)
        m["u_slab"] = np.ascontiguousarray(u_padf[:, :, ROWS * k:ROWS * k + ROWS + 2])
        in_maps.append(m)
    return in_maps


def _gather(results):
    out = np.empty((1, T1, Z1, H, W_, D), np.float32)
    for k, r in enumerate(results):
        o = r["out"]                          # [128, ROWS, 48, 48]
        out[0, :, :, ROWS * k:ROWS * (k + 1)] = o.reshape(T1, Z1, ROWS, W_, D)
    return out


_NC_CACHE = {}


def kernel(u, W, bias):
    if "nc" not in _NC_CACHE:
        _NC_CACHE["nc"] = _build_program()
    nc = _NC_CACHE["nc"]
    in_maps = _host_prep(u, W, bias)
    res = run_bass_kernel_spmd(nc, in_maps, core_ids=list(range(NCORES)))
    return _gather(res.results)


# revision 31
# speedup vs baseline: 1.0007x; 1.0007x over previous
"""Trainium2 Bass kernel for nn_CapsuleLayer (4x 3D capsule convs + dynamic routing).

Self-contained: kernel(**inputs) takes the FULL unsharded inputs and returns the
FULL output. Internally shards the 48^3 volume across 8 NeuronCores along H
(6 output rows each, 1-row halo), runs one SPMD Bass program, and gathers.

Math (exploits the reference's routing list-collapse):
  u_hat_i = relu(conv3d(u[:,i], W[i]) + bias_i)          (4 capsule convs)
  S = sum_i u_hat_i;  per (t, pos): St, SSt, s3, s0, q0  (z1-reductions)
  b1 = f(SSt/64)*St/8*s3 ; c2 = softmax_t(b1)
  b2 = b1 + f(c2^2 q0)*c2*s0^2 ; c3 = softmax_t(b2)
  out[t,z] = f(c3^2 q0)*c3 * u_hat_0[t,z],  f(s) = (s/(1+s))/sqrt(s+1e-9)

Conv layout: per output h-row, the (w,d) plane is zero-padded to 50x50 and
flattened (f = w*50 + d) so all 27 taps are free-axis offsets. Contraction is
stacked to K=128 over 8 (kh,kw) tap-pairs x z0=16 (the 9th pair runs as 3
row-tiled K=16 matmuls), with kd as a free-axis shift. All matmuls fp32
(the routing softmax is chaotic: logits span +-100, so bf16/tf32 convs are
numerically dead).
"""

import os
import numpy as np

import concourse.bass as bass
import concourse.tile as tile
from concourse import bacc
from concourse import mybir
from concourse.bass_utils import run_bass_kernel_spmd

F32 = mybir.dt.float32
AF = mybir.ActivationFunctionType
ALU = mybir.AluOpType

T0, Z0, T1, Z1 = 4, 16, 8, 16
H = W_ = D = 48
NCORES, ROWS = 8, 6          # 8 cores x 6 output h-rows
PW = 50                      # padded w/d extent
PF = 2504                    # padded flattened plane row (50*50 -> 2504)
BW = 2404                    # stacked block free width
ROWF = 2400                  # per-row output f extent (48 w-rows x 50)
TILES = [(0, 500), (500, 500), (1000, 500), (1500, 500), (2000, 400)]
CHUNK, NCHUNK = 120, 20      # transpose chunking of ROWF
PAIRS = [(a, b) for a in range(3) for b in range(3)][:8]   # leftover: (2,2)
# bisect knobs (debug only)
_ROWS_RUN = int(os.environ.get("K_ROWS", ROWS))
_CAPS_RUN = int(os.environ.get("K_CAPS", T0))
_PHASE = int(os.environ.get("K_PHASE", 9))   # 1=conv,2=+reduce,3=+chain,9=all


def _build_program():
    nc = bacc.Bacc("TRN2", target_bir_lowering=False, debug=False)

    u_slab = nc.dram_tensor("u_slab", [T0, Z0, ROWS + 2, PF], F32,
                            kind="ExternalInput").ap()
    wmain = nc.dram_tensor("wmain", [T0, 3, 128, 128], F32,
                           kind="ExternalInput").ap()
    wlft = nc.dram_tensor("wlft", [T0, 48, 128], F32,
                          kind="ExternalInput").ap()
    biasT = nc.dram_tensor("biasT", [128, T0], F32, kind="ExternalInput").ap()
    iden = nc.dram_tensor("iden", [128, 128], F32, kind="ExternalInput").ap()
    out = nc.dram_tensor("out", [128, ROWS, W_, D], F32,
                         kind="ExternalOutput").ap()
    # windowed DRAM staging: u3[cap, z, r, b, f] = u_slab[cap, z, r, b*50+f]
    u3 = nc.dram_tensor("u3", [T0, Z0, ROWS + 2, 3, BW], F32,
                        kind="Internal").ap()

    from contextlib import ExitStack
    with tile.TileContext(nc) as tc, ExitStack() as es:
        const = es.enter_context(tc.tile_pool(name="const", bufs=1))
        stkp = es.enter_context(tc.tile_pool(name="stkp", bufs=3))
        rowp = es.enter_context(tc.tile_pool(name="rowp", bufs=2))
        scrp = es.enter_context(tc.tile_pool(name="scrp", bufs=3))
        chp = es.enter_context(tc.tile_pool(name="chp", bufs=2))
        psc = es.enter_context(tc.tile_pool(name="psc", bufs=2, space="PSUM"))
        pst = es.enter_context(tc.tile_pool(name="pst", bufs=2, space="PSUM"))
        psg = es.enter_context(tc.tile_pool(name="psg", bufs=1, space="PSUM"))
        # dedicated bank for "touch" matmuls that pre-absorb DMA semaphores
        # onto the PE vector clock (the LW+MM pair encodes very few sync
        # commands, so real matmuls must arrive with their waits pre-observed)
        pstc = es.enter_context(tc.tile_pool(name="pstc", bufs=1, space="PSUM"))

        # --- constants ---
        wm = const.tile([128, T0 * 3, 128], F32)          # main lhsT per (cap,c3)
        for cap in range(T0):
            for c3 in range(3):
                nc.sync.dma_start(wm[:, cap * 3 + c3, :], wmain[cap, c3])
        wl = const.tile([48, T0, 128], F32)               # leftover lhsT (c3,z0)
        for cap in range(T0):
            nc.sync.dma_start(wl[:, cap, :], wlft[cap])
        bias = const.tile([128, T0], F32)
        nc.sync.dma_start(bias[:], biasT[:])
        idn = const.tile([128, 128], F32)
        nc.sync.dma_start(idn[:], iden[:])
        epsT = const.tile([128, 1], F32)
        nc.any.memset(epsT[:], 1e-9)

        # DRAM->DRAM window expansion (3-dim APs, one DMA per (cap, row))
        for cap in range(T0):
            for r in range(ROWS + 2):
                s = bass.AP(u_slab.tensor,
                            u_slab.offset + (cap * Z0 * 8 + r) * PF,
                            [[8 * PF, Z0], [PW, 3], [1, BW]])
                nc.sync.dma_start(u3[cap, :, r], s)

        ptc = pstc.tile([1, 16], F32)

        def touch(rhs_ap, lhsT_ap=None):
            nc.tensor.matmul(ptc[0:1, 0:2],
                             lhsT_ap if lhsT_ap is not None else idn[:, 0:1],
                             rhs_ap, start=True, stop=True,
                             skip_group_check=True)

        touch(wm[:, 0, 0:2], wm[:, 0, 0:1])
        touch(idn[:, 0:2], idn[:, 0:1])

        for h in range(_ROWS_RUN):
            U0 = rowp.tile([128, ROWF], F32, tag="U0")
            S = rowp.tile([128, ROWF], F32, tag="S")

            for cap in range(_CAPS_RUN):
                stk = stkp.tile([128, BW], F32, tag="stk")
                # pairs p=3a+b are contiguous (r*3+b = h*3+p) in u3
                s = bass.AP(u3.tensor,
                            u3.offset + (cap * Z0 * (ROWS + 2) * 3 + h * 3) * BW,
                            [[BW, 8], [(ROWS + 2) * 3 * BW, Z0], [1, BW]])
                nc.sync.dma_start(stk[:, :], s)
                lft = stkp.tile([48, BW], F32, tag="lft")
                for c3 in range(3):
                    nc.sync.dma_start(lft[Z0 * c3:Z0 * (c3 + 1), 0:BW - 2],
                                      u3[cap, :, h + 2, 2, c3:c3 + BW - 2])
                touch(stk[:, 0:2])
                touch(lft[:, 0:2], wl[:, cap, 0:1])

                uh = U0 if cap == 0 else scrp.tile([128, ROWF], F32, tag="uh")
                for off, wd in TILES:
                    ps = psc.tile([128, 512], F32, tag="conv")
                    for c3 in range(3):
                        nc.tensor.matmul(ps[:, :wd], wm[:, cap * 3 + c3, :],
                                         stk[:, off + c3:off + c3 + wd],
                                         start=(c3 == 0), stop=False)
                    nc.tensor.matmul(ps[:, :wd], wl[:, cap, :],
                                     lft[:, off:off + wd],
                                     start=False, stop=True)
                    # bias + relu (PSUM -> SBUF)
                    nc.scalar.activation(uh[:, off:off + wd], ps[:, :wd],
                                         AF.Relu, bias=bias[:, cap:cap + 1])

                    if _PHASE < 2 or cap == 0:
                        continue
                    base = U0 if cap == 1 else S
                    nc.vector.tensor_add(S[:, off:off + wd],
                                         base[:, off:off + wd],
                                         uh[:, off:off + wd])
                if cap == 3:
                    uh3 = uh

            if _PHASE < 3:
                nc.sync.dma_start(
                    out[:, h],
                    U0[:].rearrange("p (w d) -> p w d", d=PW)[:, :, 0:D])
                continue
            # --- transpose S/U0/uh3 to spatial-major, reduce z1 on DVE ---
            Rs0 = chp.tile([CHUNK, NCHUNK, 8], F32, tag="Rs0")
            Rq0 = chp.tile([CHUNK, NCHUNK, 8], F32, tag="Rq0")
            Rst = chp.tile([CHUNK, NCHUNK, 8], F32, tag="Rst")
            Rss = chp.tile([CHUNK, NCHUNK, 8], F32, tag="Rss")
            Rs3 = chp.tile([CHUNK, NCHUNK, 8], F32, tag="Rs3")
            for g in range(NCHUNK // 2):          # 2 chunks per group
                tp = pst.tile([CHUNK, 2, 512], F32, tag="tp")
                for j in range(2):
                    c = 2 * g + j
                    sl = slice(CHUNK * c, CHUNK * (c + 1))
                    nc.tensor.transpose(tp[:, j, 0:128], S[:, sl], idn)
                    nc.tensor.transpose(tp[:, j, 128:256], U0[:, sl], idn)
                    nc.tensor.transpose(tp[:, j, 256:384], uh3[:, sl], idn)
                gsl = slice(2 * g, 2 * g + 2)
                qsc = chp.tile([CHUNK, 2, 128], F32, tag="qsc")
                for (R, base) in ((Rst, 0), (Rs0, 128), (Rs3, 256)):
                    nc.vector.tensor_reduce(
                        R[:, gsl, :],
                        tp[:, :, base:base + 128].rearrange(
                            "p g (t z) -> p g t z", z=Z1),
                        mybir.AxisListType.X, ALU.add)
                for (R, base) in ((Rss, 0), (Rq0, 128)):
                    nc.scalar.activation(qsc[:], tp[:, :, base:base + 128],
                                         AF.Square)
                    nc.vector.tensor_reduce(
                        R[:, gsl, :],
                        qsc[:].rearrange("p g (t z) -> p g t z", z=Z1),
                        mybir.AxisListType.X, ALU.add)

            s0v = Rs0[:]
            q0v = Rq0[:]
            s3v = Rs3[:]

            sh8 = [CHUNK, NCHUNK, 8]
            sh1 = [CHUNK, NCHUNK, 1]

            def t8(tag):
                return chp.tile(sh8, F32, tag=tag, name=tag)

            def t1(tag):
                return chp.tile(sh1, F32, tag=tag, name=tag)

            def fchain(sqv, tagp):
                """returns tile containing f(sqv) = sqv/((1+sqv)*sqrt(sqv+1e-9))"""
                r = t8(tagp + "r")
                nc.scalar.activation(r[:], sqv, AF.Sqrt, bias=epsT[0:CHUNK, :])
                d = t8(tagp + "d")
                nc.vector.tensor_scalar_add(d[:], sqv, 1.0)
                nc.vector.tensor_mul(d[:], d[:], r[:])
                rc = t8(tagp + "rc")
                nc.vector.reciprocal(rc[:], d[:])
                f = t8(tagp + "f")
                nc.vector.tensor_mul(f[:], sqv, rc[:])
                return f

            def softmax(b, tagp):
                m = t1(tagp + "m")
                nc.vector.tensor_reduce(m[:], b[:], mybir.AxisListType.X, ALU.max)
                e = t8(tagp + "e")
                nc.vector.tensor_sub(e[:], b[:], m[:].broadcast_to(sh8))
                nc.scalar.activation(e[:], e[:], AF.Exp)
                dn = t1(tagp + "dn")
                nc.vector.tensor_reduce(dn[:], e[:], mybir.AxisListType.X, ALU.add)
                rd = t1(tagp + "rd")
                nc.vector.reciprocal(rd[:], dn[:])
                c = t8(tagp + "c")
                nc.vector.tensor_mul(c[:], e[:], rd[:].broadcast_to(sh8))
                return c

            sq1 = t8("sq1")
            nc.vector.tensor_scalar_mul(sq1[:], Rss[:], 1.0 / 64.0)
            f1 = fchain(sq1[:], "f1")
            b1 = t8("b1")
            nc.vector.scalar_tensor_tensor(b1[:], f1[:], 0.125, Rst[:],
                                           ALU.mult, ALU.mult)
            nc.vector.tensor_mul(b1[:], b1[:], s3v)
            c2 = softmax(b1, "s1")
            sq2 = t8("sq2")
            nc.vector.tensor_mul(sq2[:], c2[:], c2[:])
            nc.vector.tensor_mul(sq2[:], sq2[:], q0v)
            f2 = fchain(sq2[:], "f2")
            nc.vector.tensor_mul(f2[:], f2[:], c2[:])
            nc.vector.tensor_mul(f2[:], f2[:], s0v)
            nc.vector.tensor_mul(f2[:], f2[:], s0v)
            b2 = t8("b2")
            nc.vector.tensor_add(b2[:], b1[:], f2[:])
            c3 = softmax(b2, "s2")
            sq3 = t8("sq3")
            nc.vector.tensor_mul(sq3[:], c3[:], c3[:])
            nc.vector.tensor_mul(sq3[:], sq3[:], q0v)
            g = fchain(sq3[:], "f3")
            nc.vector.tensor_mul(g[:], g[:], c3[:])

            # replicate g over z1: [120, 20, 8] -> [120, 20, 8, 16]
            grep = chp.tile([CHUNK, NCHUNK, 128], F32, tag="grep")
            nc.vector.tensor_copy(
                grep[:].rearrange("p n (t z) -> p n t z", z=Z1),
                g[:].unsqueeze(3).broadcast_to([CHUNK, NCHUNK, 8, Z1]))

            # transpose back per chunk and scale u_hat_0
            stage = rowp.tile([128, ROWF], F32, tag="stage")
            for c in range(NCHUNK):
                gb = psg.tile([128, CHUNK], F32, tag="gb")
                nc.tensor.transpose(gb[:], grep[:, c, :], idn[0:CHUNK, 0:CHUNK])
                nc.vector.tensor_mul(stage[:, CHUNK * c:CHUNK * (c + 1)],
                                     gb[:], U0[:, CHUNK * c:CHUNK * (c + 1)])

            nc.sync.dma_start(
                out[:, h],
                stage[:].rearrange("p (w d) -> p w d", d=PW)[:, :, 0:D])

    nc.compile()
    return nc


def _host_prep(u, W, bias):
    """Returns (per-core in_maps list, shared tensors dict)."""
    u = np.ascontiguousarray(u.astype(np.float32))
    W = np.ascontiguousarray(W.astype(np.float32))
    bias = np.ascontiguousarray(bias.astype(np.float32))

    u_pad = np.zeros((T0, Z0, H + 2, PW, PW), np.float32)
    u_pad[:, :, 1:-1, 1:-1, 1:-1] = u[0]
    u_pad = u_pad.reshape(T0, Z0, H + 2, PW * PW)
    u_padf = np.zeros((T0, Z0, H + 2, PF), np.float32)
    u_padf[:, :, :, :PW * PW] = u_pad

    wmain = np.zeros((T0, 3, 128, 128), np.float32)
    for p, (a, b) in enumerate(PAIRS):
        # lhsT[(16p+z0), co] = W[cap, co, z0, a, b, c3]
        wmain[:, :, Z0 * p:Z0 * (p + 1), :] = W[:, :, :, a, b, :].transpose(
            0, 3, 2, 1)
    # leftover lhsT [(c3, z0), co], c3-major to match the pre-shifted data
    wlft = W[:, :, :, 2, 2, :].transpose(0, 3, 2, 1).reshape(T0, 48, 128).copy()

    biasT = bias.T.copy()                     # [128, T0]
    iden = np.eye(128, dtype=np.float32)

    shared = {"wmain": wmain, "wlft": wlft,
              "biasT": biasT, "iden": iden}
    in_maps = []
    for k in range(NCORES):
        m = dict(shared)
        m["u_slab"] = np.ascontiguousarray(u_padf[:, :, ROWS * k:ROWS * k + ROWS + 2])
        in_maps.append(m)
    return in_maps


def _gather(results):
    out = np.empty((1, T1, Z1, H, W_, D), np.float32)
    for k, r in enumerate(results):
        o = r["out"]                          # [128, ROWS, 48, 48]
        out[0, :, :, ROWS * k:ROWS * (k + 1)] = o.reshape(T1, Z1, ROWS, W_, D)
    return out


_NC_CACHE = {}


def kernel(u, W, bias):
    if "nc" not in _NC_CACHE:
        _NC_CACHE["nc"] = _build_program()
    nc = _NC_CACHE["nc"]
    in_maps = _host_prep(u, W, bias)
    res = run_bass_kernel_spmd(nc, in_maps, core_ids=list(range(NCORES)))
    return _gather(res.results)
